# revision 32
# baseline (speedup 1.0000x reference)
"""Trainium2 Bass kernel for nn_CapsuleModel2 (capsule routing head).

Strategy (data-parallel, one image per NeuronCore, 8 cores):

Host-side folding:
  1x1conv(poses) -> vote conv -> positional linear collapses into
     tokens[(n,v), j] = Weff @ feat_pts + pegrid_pts
  AND the point gather is hoisted to the host: feat columns arrive
  already permuted into point order (feat_pts[:, j] = feat[:, sidx[j]]),
  so the device computes tokens/acts DIRECTLY per point — no GPSIMD
  gather. Row 1280 of the feat input is a mask row (-30 for masked
  points) accumulated into the act logits via a 1-partition matmul.

Device pipeline per core (one image):
  G:  tokens[(n,v), j] (bf16, DVE add of psum+pegrid), act logits
      z[n, j] (+mask row), exp(-z-b) written straight into a
      4x32-partition-block quartered layout; 3 DVE ops finish
      sg = sigmoid+1e-6 as [128, 1024] bf16.
  R1: per 128-point chunk: L = tok.T @ blockdiag(Q1/4), E = exp(L) bf16;
      vals = sg * (tok.T @ blockdiag(Wv1)) bf16; numerator matmuls in
      the [o=64, j=17] orientation so all 16 instances accumulate into
      ONE psum bank [128, 17*8] and R2 runs fully batched.
  R2: batched across all 16 instances on [128,*]/[19,272] tiles:
      normalize, a1 sigmoid, Q2 attention, class sigmoid -> out [19,16].

Inputs are packed into 3 DRAM tensors (feat, cf32, cbf16) to minimize
per-dispatch argument overhead.

Hardware pitfall encoded here: back-to-back PE matmuls whose operand
base partition flips 0<->64 lock up the device (probed on HW), so the
R2 class-attention stage DMA-shifts odd instances to partition base 0.
"""

import sys

for _p in ("/opt/trn_rl_repo",):
    if _p not in sys.path:
        sys.path.insert(0, _p)

import numpy as np
import ml_dtypes

import concourse.bacc as bacc
import concourse.tile as tile
from concourse import mybir
from concourse import bass_utils

AF = mybir.ActivationFunctionType
ALU = mybir.AluOpType
F32 = mybir.dt.float32
BF16 = mybir.dt.bfloat16
FP8 = mybir.dt.float8e4
PM = mybir.MatmulPerfMode
BF16_NP = ml_dtypes.bfloat16
FP8_NP = ml_dtypes.float8_e4m3

B, I, P = 8, 16, 256
CIN = 1280
NCAPS, DCAP, DV = 8, 32, 16
HF = WF = 64
S = HF * WF              # 4096 grid positions
NPTS = I * P             # 4096 points (== S by coincidence)
NOUT1, NCLS = 64, 19
KT = CIN // 128          # 10 contraction tiles
HALF = NPTS // 2
NCH = 32                 # routing-1 chunks of 128 points

# --- cfp8 blob column offsets (fp8 e4m3; DoubleRow pairs) ---
OQ_WEFF = 0                      # [128, 5*2*128] weffT double-row pairs
OQ_WA = OQ_WEFF + KT * 128       # [128, 5*2*16]  waT double-row pairs (padded)
W_FP8 = OQ_WA + KT * 16          # 1440

# --- cbf16 blob column offsets (bf16) ---
OB_BQ1 = 0                       # [128, 512]    blockdiag Q1/4
OB_BWV = OB_BQ1 + 512            # [128, 136]    blockdiag Wv1 (col16=0)
OB_E8 = OB_BWV + 136             # [128, 136]    act-replication matrix
OB_Q2 = OB_E8 + 136              # [16, 20]      (Q2/4).T zero-padded
OB_IDT = OB_Q2 + 20              # [128, 128]    identity (transposes)
OB_ONE = OB_IDT + 128            # [1, 16]       ones (mask-row matmul)
W_BF16 = OB_ONE + 16             # 948

# --- cf32 blob column offsets (f32; pegrid part is per-image) ---
OF_PEG = 0                       # [128, 4096] positional table at points
OF_W1R = OF_PEG + NPTS           # [128, 136]  wact1 tiled per block, col16=0
OF_W2R = OF_W1R + 136            # [19, 272]   wact2 tiled per block, col16=0
OF_NB1 = OF_W2R + 272            # [128, 1]    -bact1
OF_NB2 = OF_NB1 + 1              # [19, 1]     -bact2
OF_NBG = OF_NB2 + 1              # [128, 1]    row 32m+n = -bacts[n]
W_F32 = OF_NBG + 1               # 4507

_CACHE = {}


def _build_nc(repeat=1, phases=4):
    nc = bacc.Bacc("TRN2", target_bir_lowering=False, debug=False, num_devices=8)

    feat = nc.dram_tensor("feat", [CIN + 1, NPTS], FP8, kind="ExternalInput").ap()
    cfp8 = nc.dram_tensor("cfp8", [128, W_FP8], FP8, kind="ExternalInput").ap()
    cf32 = nc.dram_tensor("cf32", [128, W_F32], F32, kind="ExternalInput").ap()
    cbf16 = nc.dram_tensor("cbf16", [128, W_BF16], BF16, kind="ExternalInput").ap()
    out_cls = nc.dram_tensor("out_cls", [I, NCLS], F32, kind="ExternalOutput").ap()

    with tile.TileContext(nc) as tc:
        with (
            tc.tile_pool(name="cons", bufs=1) as cons,
            tc.tile_pool(name="grid", bufs=1) as grid,
            tc.tile_pool(name="feats", bufs=4) as feats,
            tc.tile_pool(name="rsb", bufs=6) as rsb,
            tc.tile_pool(name="small", bufs=1) as small,
        ):
            # ---- constants via the gpsimd (SWDGE) queue; feat on sync ----
            cb = cons.tile([128, W_BF16], BF16)
            nc.gpsimd.dma_start(out=cb[:], in_=cbf16)
            c8 = cons.tile([128, W_FP8], FP8)
            nc.gpsimd.dma_start(out=c8[:], in_=cfp8)
            cf = cons.tile([128, W_F32], F32)
            for jq in range(2):
                half = W_F32 // 2
                lo = jq * half
                hi = W_F32 if jq == 1 else half
                nc.gpsimd.dma_start(out=cf[:, lo:hi], in_=cf32[:, lo:hi])

            # ---- persistent tensors ----
            tokens_sb = grid.tile([128, NPTS], BF16)
            zq = grid.tile([128, NPTS // 4], F32)    # quartered act logits
            sg2 = grid.tile([128, NPTS // 4], BF16)  # sigmoid(z)+1e-6
            outcls_sb = grid.tile([NCLS, I], F32)
            nc.vector.memset(zq[:, :], 0.0)

            for rep in range(repeat):
                # ================= phase G: point matmuls ================
                with tc.tile_pool(name=f"pg{rep}", bufs=1, space="PSUM") as pg:
                    for h in range(2):
                        mrow = feats.tile([1, HALF], FP8, tag="mrow", name=f"mr{h}_{rep}")
                        nc.sync.dma_start(
                            out=mrow[:], in_=feat[CIN : CIN + 1, h * HALF : (h + 1) * HALF]
                        )
                        pms = [
                            pg.tile([128, 512], F32, tag=f"pm{nn}", name=f"pm{nn}_{h}_{rep}")
                            for nn in range(4)
                        ]
                        pas = [
                            pg.tile([16, 512], F32, tag=f"pa{nn}", name=f"pa{nn}_{h}_{rep}")
                            for nn in range(4)
                        ]
                        for k2 in range(KT // 2):
                            ft = feats.tile([128, 2 * HALF], FP8, tag="feat")
                            for two in range(2):
                                nc.sync.dma_start(
                                    out=ft[:, two * HALF : (two + 1) * HALF],
                                    in_=feat[
                                        k2 * 256 + two * 128 : k2 * 256 + (two + 1) * 128,
                                        h * HALF : (h + 1) * HALF,
                                    ],
                                )
                            ftv = ft[:].rearrange("p (two c) -> p two c", two=2)
                            wm = c8[
                                :, OQ_WEFF + k2 * 256 : OQ_WEFF + (k2 + 1) * 256
                            ].rearrange("p (two m) -> p two m", two=2)
                            wa = c8[
                                :, OQ_WA + k2 * 32 : OQ_WA + (k2 + 1) * 32
                            ].rearrange("p (two m) -> p two m", two=2)
                            for nn in range(4):
                                nc.tensor.matmul(
                                    pms[nn][:],
                                    lhsT=wm,
                                    rhs=ftv[:, :, nn * 512 : (nn + 1) * 512],
                                    start=(k2 == 0),
                                    stop=(k2 == KT // 2 - 1),
                                    perf_mode=PM.DoubleRow,
                                )
                            for nn in range(4):
                                nc.tensor.matmul(
                                    pas[nn][:],
                                    lhsT=wa,
                                    rhs=ftv[:, :, nn * 512 : (nn + 1) * 512],
                                    start=(k2 == 0),
                                    stop=False,
                                    perf_mode=PM.DoubleRow,
                                )
                        # mask row folds into the act logits
                        for nn in range(4):
                            nc.tensor.matmul(
                                pas[nn][:],
                                lhsT=cb[0:1, OB_ONE : OB_ONE + 16],
                                rhs=mrow[:, nn * 512 : (nn + 1) * 512],
                                start=False,
                                stop=True,
                            )
                        for nn in range(4):
                            off = h * HALF + nn * 512
                            q, loc = off // 1024, off % 1024
                            nc.vector.tensor_add(
                                out=tokens_sb[:, off : off + 512],
                                in0=pms[nn][:],
                                in1=cf[:, OF_PEG + off : OF_PEG + off + 512],
                            )
                            # exp(-z - bacts) straight into the quartered
                            # 32-row-block layout R1's broadcast matmul eats
                            nc.scalar.activation(
                                out=zq[32 * q : 32 * q + 8, loc : loc + 512],
                                in_=pas[nn][0:8, :],
                                func=AF.Exp,
                                scale=-1.0,
                                bias=cf[32 * q : 32 * q + 8, OF_NBG : OF_NBG + 1],
                            )

                if phases < 2:
                    nc.vector.memset(outcls_sb[:, :], 0.0)
                    continue

                # finish sg = 1/(1+exp(-z-b)) + 1e-6 on [128, 1024]
                nc.vector.tensor_scalar_add(out=zq[:], in0=zq[:], scalar1=1.0)
                nc.vector.reciprocal(out=zq[:], in_=zq[:])
                nc.vector.tensor_scalar_add(out=sg2[:], in0=zq[:], scalar1=1e-6)

                if phases < 3:
                    nc.vector.memset(outcls_sb[:, :], 0.0)
                    continue

                # ================= phase R1: routing 1 ===================
                with (
                    tc.tile_pool(name=f"pn{rep}", bufs=1, space="PSUM") as pnp,
                ):
                    pn_all = pnp.tile([128, 136], F32, name=f"pnall_{rep}")

                    with (
                        tc.tile_pool(name=f"pl{rep}", bufs=3, space="PSUM") as plp,
                        tc.tile_pool(name=f"pv{rep}", bufs=3, space="PSUM") as pvp,
                    ):
                        def emit_front(c):
                            tokc = tokens_sb[:, c * 128 : (c + 1) * 128]
                            m4 = c // 8
                            pl = plp.tile([128, 512], F32, tag="pl", name=f"pl{c}_{rep}")
                            nc.tensor.matmul(
                                pl[:], lhsT=tokc,
                                rhs=cb[:, OB_BQ1 : OB_BQ1 + 512],
                                start=True, stop=True,
                            )
                            E = rsb.tile([128, 512], BF16, tag="E", name=f"E{c}_{rep}")
                            nc.scalar.activation(out=E[:], in_=pl[:], func=AF.Exp)

                            pvpa = pvp.tile([128, 272], F32, tag="pv", name=f"pv{c}_{rep}")
                            nc.tensor.matmul(
                                pvpa[:, 0:136], lhsT=tokc,
                                rhs=cb[:, OB_BWV : OB_BWV + 136],
                                start=True, stop=True,
                            )
                            nc.tensor.matmul(
                                pvpa[:, 136:272],
                                lhsT=sg2[
                                    32 * m4 : 32 * m4 + 8,
                                    (c % 8) * 128 : (c % 8 + 1) * 128,
                                ],
                                rhs=cb[32 * m4 : 32 * m4 + 8, OB_E8 : OB_E8 + 136],
                                start=True, stop=True,
                                tile_position=(32 * m4, 0),
                            )
                            asig = rsb.tile([128, 136], BF16, tag="asig", name=f"as{c}_{rep}")
                            nc.vector.tensor_copy(out=asig[:], in_=pvpa[:, 136:272])
                            vals = rsb.tile([128, 136], BF16, tag="vals", name=f"va{c}_{rep}")
                            nc.vector.tensor_mul(
                                out=vals[:], in0=pvpa[:, 0:136], in1=asig[:]
                            )
                            vr = vals[:].rearrange("p (n j) -> p n j", j=17)
                            ar = asig[:].rearrange("p (n j) -> p n j", j=17)
                            nc.vector.tensor_copy(
                                out=vr[:, :, 16:17], in_=ar[:, :, 16:17]
                            )
                            return E, vals

                        def emit_numer(c, E, vals):
                            i = c // 2
                            po = 64 * (i % 2)
                            co = 17 * (i // 2)
                            for n in range(8):
                                nc.tensor.matmul(
                                    pn_all[po : po + 64, co : co + 17],
                                    lhsT=E[:, n * 64 : (n + 1) * 64],
                                    rhs=vals[:, n * 17 : (n + 1) * 17],
                                    start=(c % 2 == 0 and n == 0),
                                    stop=(c % 2 == 1 and n == 7),
                                    skip_group_check=True,
                                )

                        from collections import deque
                        pending = deque()
                        for c in range(NCH):
                            front = emit_front(c)
                            pending.append((c,) + front)
                            if len(pending) > 2:
                                emit_numer(*pending.popleft())
                        while pending:
                            emit_numer(*pending.popleft())

                    # ================= phase R2: batched =================
                    if phases < 4:
                        nc.vector.memset(outcls_sb[:, :], 0.0)
                        continue
                    with (
                        tc.tile_pool(name=f"pt{rep}", bufs=1, space="PSUM") as ptp,
                        tc.tile_pool(name=f"pq{rep}", bufs=1, space="PSUM") as pqp,
                    ):
                        pnS = small.tile([128, 136], F32, name=f"pnS_{rep}")
                        nc.vector.tensor_copy(out=pnS[:], in_=pn_all[:])
                        pnV = pnS[:].rearrange("p (i j) -> p i j", j=17)

                        recd = small.tile([128, 8], F32, name=f"recd_{rep}")
                        nc.vector.reciprocal(out=recd[:], in_=pnV[:, :, 16:17])

                        z1t = small.tile([128, 136], F32, name=f"z1t_{rep}")
                        nc.vector.tensor_mul(
                            out=z1t[:], in0=pnS[:], in1=cf[:, OF_W1R : OF_W1R + 136]
                        )
                        s1 = small.tile([128, 8], F32, name=f"s1_{rep}")
                        nc.vector.reduce_sum(
                            out=s1[:],
                            in_=z1t[:].rearrange("p (i j) -> p i j", j=17),
                            axis=mybir.AxisListType.X,
                        )
                        z1 = small.tile([128, 8], F32, name=f"z1_{rep}")
                        nc.vector.tensor_mul(out=z1[:], in0=s1[:], in1=recd[:])
                        a1e = small.tile([128, 8], F32, name=f"a1e_{rep}")
                        nc.scalar.activation(
                            out=a1e[:], in_=z1[:], func=AF.Exp, scale=-1.0,
                            bias=cf[:, OF_NB1 : OF_NB1 + 1],
                        )
                        nc.vector.tensor_scalar_add(out=a1e[:], in0=a1e[:], scalar1=1.0)
                        nc.vector.reciprocal(out=a1e[:], in_=a1e[:])
                        nc.vector.tensor_scalar_add(out=a1e[:], in0=a1e[:], scalar1=1e-6)
                        sc = small.tile([128, 8], F32, name=f"sc_{rep}")
                        nc.vector.tensor_mul(out=sc[:], in0=recd[:], in1=a1e[:])

                        p1b = small.tile([128, 128], BF16, name=f"p1b_{rep}")
                        # 20-col blocks keep pnd's rhs slices 4B-aligned
                        pv2 = small.tile([128, 160], BF16, name=f"pv2_{rep}")
                        nc.vector.memset(pv2[:, :], 0.0)
                        for k in range(8):
                            nc.vector.tensor_scalar_mul(
                                out=p1b[:, 16 * k : 16 * k + 16],
                                in0=pnS[:, 17 * k : 17 * k + 16],
                                scalar1=recd[:, k : k + 1],
                            )
                            nc.vector.tensor_scalar_mul(
                                out=pv2[:, 20 * k : 20 * k + 16],
                                in0=pnS[:, 17 * k : 17 * k + 16],
                                scalar1=sc[:, k : k + 1],
                            )
                        pv2V = pv2[:].rearrange("p (i j) -> p i j", j=20)
                        nc.vector.tensor_copy(out=pv2V[:, :, 16:17], in_=a1e[:])
                        # odd instances DMA-shifted to partition base 0 early
                        # (operand base flipping 0<->64 locks up the PE)
                        pv2lo = small.tile([64, 160], BF16, name=f"pv2lo_{rep}")
                        nc.sync.dma_start(out=pv2lo[:], in_=pv2[64:128, :])

                        # transposes: p1 [128,16] blocks -> [16,128] (2 inst)
                        pts = [
                            ptp.tile([16, 512], BF16, tag=f"pt{t}", name=f"pt{t}_{rep}")
                            for t in range(2)
                        ]
                        for k in range(8):
                            nc.tensor.transpose(
                                out=pts[k // 4][:, 128 * (k % 4) : 128 * (k % 4) + 128],
                                in_=p1b[:, 16 * k : 16 * k + 16],
                                identity=cb[:, OB_IDT : OB_IDT + 128],
                            )
                        pTS = small.tile([16, 1024], BF16, name=f"pTS_{rep}")
                        for t in range(2):
                            nc.vector.tensor_copy(
                                out=pTS[:, 512 * t : 512 * t + 512], in_=pts[t][:]
                            )

                        pL2 = pqp.tile([128, 160], F32, name=f"pL2_{rep}")
                        for k in range(8):
                            nc.tensor.matmul(
                                pL2[:, 20 * k : 20 * k + 20],
                                lhsT=pTS[:, 128 * k : 128 * k + 128],
                                rhs=cb[0:16, OB_Q2 : OB_Q2 + 20],
                                start=True, stop=True,
                            )
                        E2 = small.tile([128, 160], BF16, name=f"E2_{rep}")
                        nc.scalar.activation(out=E2[:], in_=pL2[:], func=AF.Exp)
                        E2lo = small.tile([64, 160], BF16, name=f"E2lo_{rep}")
                        nc.sync.dma_start(out=E2lo[:], in_=E2[64:128, :])
                        pnd = pqp.tile([NCLS, 272], F32, name=f"pnd_{rep}")
                        for i in range(I):
                            k, hh = i // 2, i % 2
                            Esrc = E2 if hh == 0 else E2lo
                            vsrc = pv2 if hh == 0 else pv2lo
                            nc.tensor.matmul(
                                pnd[:, 17 * i : 17 * i + 17],
                                lhsT=Esrc[0:64, 20 * k : 20 * k + 19],
                                rhs=vsrc[0:64, 20 * k : 20 * k + 17],
                                start=True, stop=True,
                            )
                        pndS = small.tile([NCLS, 272], F32, name=f"pndS_{rep}")
                        nc.vector.tensor_copy(out=pndS[:], in_=pnd[:])
                        pndV = pndS[:].rearrange("p (i j) -> p i j", j=17)
                        recd2 = small.tile([NCLS, 16], F32, name=f"recd2_{rep}")
                        nc.vector.reciprocal(out=recd2[:], in_=pndV[:, :, 16:17])
                        z2t = small.tile([NCLS, 272], F32, name=f"z2t_{rep}")
                        nc.vector.tensor_mul(
                            out=z2t[:], in0=pndS[:], in1=cf[0:NCLS, OF_W2R : OF_W2R + 272]
                        )
                        s2 = small.tile([NCLS, 16], F32, name=f"s2_{rep}")
                        nc.vector.reduce_sum(
                            out=s2[:],
                            in_=z2t[:].rearrange("p (i j) -> p i j", j=17),
                            axis=mybir.AxisListType.X,
                        )
                        z2 = small.tile([NCLS, 16], F32, name=f"z2_{rep}")
                        nc.vector.tensor_mul(out=z2[:], in0=s2[:], in1=recd2[:])
                        ez2 = small.tile([NCLS, 16], F32, name=f"ez2_{rep}")
                        nc.scalar.activation(
                            out=ez2[:], in_=z2[:], func=AF.Exp, scale=-1.0,
                            bias=cf[0:NCLS, OF_NB2 : OF_NB2 + 1],
                        )
                        nc.vector.tensor_scalar_add(out=ez2[:], in0=ez2[:], scalar1=1.0)
                        nc.vector.reciprocal(out=outcls_sb[:], in_=ez2[:])

            nc.sync.dma_start(out=out_cls.rearrange("i c -> c i"), in_=outcls_sb[:])

    nc.compile()
    return nc


def _get_nc():
    if "nc" not in _CACHE:
        _CACHE["nc"] = _build_nc()
    return _CACHE["nc"]


def host_prep(inputs):
    """Build the per-core input maps (all numpy, host-side weight folding
    plus the point-gather of feat columns)."""
    f8 = np.float64
    w_pos = np.asarray(inputs["w_pos"], f8)          # (16, 18)
    W16 = w_pos[:, :16]
    w_d = w_pos[:, 16] - w_pos[:, 17]                # (16,)
    b_pos = np.asarray(inputs["b_pos"], f8)
    w_vote = np.asarray(inputs["w_vote"], f8)        # (8, 16, 32)
    b_vote = np.asarray(inputs["b_vote"], f8)        # (8, 16)
    Wp = np.asarray(inputs["w_poses"], f8).reshape(NCAPS, DCAP, CIN)
    b_poses = np.asarray(inputs["b_poses"], f8).reshape(NCAPS, DCAP)

    Weff = np.stack([W16 @ w_vote[n] @ Wp[n] for n in range(NCAPS)])  # (8,16,1280)
    beff = np.stack(
        [W16 @ (w_vote[n] @ b_poses[n] + b_vote[n]) + b_pos for n in range(NCAPS)]
    )
    Weff = Weff.reshape(128, CIN)
    beff = beff.reshape(128)
    wd_rep = np.tile(w_d, NCAPS)                     # (128,)

    Q1s = np.asarray(inputs["Q1"], f8) / 4.0         # (64, 16)
    BQ1 = np.zeros((128, 512), f8)
    for n in range(NCAPS):
        BQ1[n * 16 : (n + 1) * 16, n * 64 : (n + 1) * 64] = Q1s.T
    Wv1 = np.asarray(inputs["Wv1"], f8)
    BWV1 = np.zeros((128, 136), f8)
    for n in range(NCAPS):
        BWV1[n * 16 : (n + 1) * 16, n * 17 : n * 17 + 16] = Wv1
    EXP8REP = np.zeros((128, 136), f8)
    for m in range(4):
        for n in range(NCAPS):
            EXP8REP[32 * m + n, n * 17 : (n + 1) * 17] = 1.0

    # ---- cfp8 blob (DoubleRow pair layout: [p, k2, two, m]) ----
    cfp8 = np.zeros((128, W_FP8), np.float64)
    # wd[p, k2, two, m] = Weff[m, (2*k2+two)*128 + p]
    wd = Weff.T.reshape(KT // 2, 2, 128, 128).transpose(2, 0, 1, 3)
    cfp8[:, OQ_WEFF : OQ_WEFF + KT * 128] = wd.reshape(128, KT * 128)
    wa8 = np.zeros((KT // 2, 2, 128, 16))
    wa8[:, :, :, 0:8] = np.asarray(inputs["w_acts"], f8).T.reshape(KT // 2, 2, 128, 8)
    cfp8[:, OQ_WA : OQ_WA + KT * 16] = wa8.transpose(2, 0, 1, 3).reshape(128, KT * 16)
    cfp8 = cfp8.astype(FP8_NP)

    # ---- cbf16 blob ----
    cbf16 = np.zeros((128, W_BF16), np.float64)
    cbf16[:, OB_BQ1 : OB_BQ1 + 512] = BQ1
    cbf16[:, OB_BWV : OB_BWV + 136] = BWV1
    cbf16[:, OB_E8 : OB_E8 + 136] = EXP8REP
    cbf16[0:16, OB_Q2 : OB_Q2 + NCLS] = (np.asarray(inputs["Q2"], f8) / 4.0).T
    cbf16[:, OB_IDT : OB_IDT + 128] = np.eye(128)
    cbf16[0:1, OB_ONE : OB_ONE + 16] = 1.0
    cbf16 = cbf16.astype(BF16_NP)

    # ---- cf32 blob (shared part; pegrid filled per image) ----
    cf32_base = np.zeros((128, W_F32), np.float32)
    wact1 = np.asarray(inputs["wact1"], np.float64)
    w1row = np.tile(np.concatenate([wact1, [0.0]]), NCAPS)        # (136,)
    cf32_base[:, OF_W1R : OF_W1R + 136] = w1row[None, :]
    wact2 = np.asarray(inputs["wact2"], np.float64)
    w2row = np.tile(np.concatenate([wact2, [0.0]]), I)            # (272,)
    cf32_base[0:NCLS, OF_W2R : OF_W2R + 272] = w2row[None, :]
    cf32_base[:, OF_NB1] = -float(np.asarray(inputs["bact1"]))
    cf32_base[0:NCLS, OF_NB2] = -float(np.asarray(inputs["bact2"]))
    bacts = np.asarray(inputs["b_acts"], np.float64)
    for m in range(4):
        cf32_base[32 * m : 32 * m + 8, OF_NBG] = -bacts

    feats = np.asarray(inputs["feature_output"])     # (8, 1280, 64, 64) f32
    coords = np.asarray(inputs["point_coords"])      # (8, 16, 2, 256) int32
    mask = np.asarray(inputs["point_mask"])          # (8, 16, 256) bool

    in_maps = []
    for b in range(B):
        y = np.clip(coords[b, :, 0, :], 0, HF - 1).astype(np.int64)
        x = np.clip(coords[b, :, 1, :], 0, WF - 1).astype(np.int64)
        sidx = (y * WF + x).reshape(NPTS)
        mb = mask[b].reshape(NPTS)

        fb = feats[b].reshape(CIN, S)
        feat_pts = np.empty((CIN + 1, NPTS), FP8_NP)
        feat_pts[0:CIN] = fb[:, sidx].astype(FP8_NP)
        feat_pts[CIN] = np.where(mb, 0.0, -30.0).astype(FP8_NP)

        yr = coords[b, :, 0, :].astype(np.float64).reshape(NPTS)
        xr = coords[b, :, 1, :].astype(np.float64).reshape(NPTS)
        r = (yr - xr) / 128.0
        cf32 = cf32_base.copy()
        cf32[:, OF_PEG : OF_PEG + NPTS] = (
            wd_rep[:, None] * r[None, :] + beff[:, None]
        ).astype(np.float32)

        in_maps.append(dict(feat=feat_pts, cf32=cf32, cbf16=cbf16, cfp8=cfp8))
    return in_maps


def kernel(**inputs):
    nc = _get_nc()
    in_maps = host_prep(inputs)
    res = bass_utils.run_bass_kernel_spmd(nc, in_maps, core_ids=list(range(B)))
    out = np.stack([np.asarray(res.results[b]["out_cls"]) for b in range(B)])
    return out.astype(np.float32)
